# revision 3
# baseline (speedup 1.0000x reference)
"""depth_to_space (DCR, block=2) on 8 NeuronCores.

out[b, 2h+i, 2w+j, c] = in[b, h, w, (2i+j)*64 + c]   for in [32,64,64,256] f32.

Sharding: batch dim B=32 split as 4 examples per core (data parallel, no
communication).

Per-core kernel: the permutation collapses to strided DRAM->DRAM DMA copies,
one per output-row parity i in {0,1}:
  - fuse (j,c) -> jc in [0,128): for fixed i the source slice
    x[:, :, :, i*128:(i+1)*128] merges (b,h,w) into a single stride dim:
    [[256, b*h*w], [1, 128]] (512B contiguous runs, 1KB stride);
  - the destination y[:, i::2, :, :] merges to [[16384, b*h], [1, 8192]]
    (output rows are fully contiguous).
No SBUF, no compute engines - pure DMA.

Engine assignment (measured via loop-in-NEFF wall-diff timing on HW): the
i=0 stream runs on the SP HWDGE ring and the i=1 stream on the Activation
HWDGE ring, each split into K=16 chunks with cross-engine semaphore waits
(lookahead D=2 chunks) so the two descriptor streams stay address-aligned.
The two streams read complementary 512B halves of each 1KB input line; when
aligned, the combined HBM read stream is dense and the copy runs at the same
rate as a contiguous D2D memcpy of equal volume (within 0.4%), i.e. at the
practical HBM roofline. That wall is session-dependent (allocation luck):
~101-103 us/core (~330 GB/s HBM R+W) in most sessions, 76 us/core
(441 GB/s) in sessions that land on unshared physical cores. Unaligned
dual-ring (~128 us), the old HWDGE+SWDGE pairing (~114 us), and a single
serialized ring (~140 us) are all slower; descriptor size (512B vs 64KB)
measures as free when the access pattern is dense.
"""

import numpy as np

import concourse.bass as bass
import concourse.mybir as mybir
from concourse.bass_utils import run_bass_kernel_spmd

B, H, W, C = 32, 64, 64, 256
KS = 2
OC = C // (KS * KS)
N_CORES = 8
BS = B // N_CORES

K_CHUNKS = 16
LOOKAHEAD = 2

_nc_cache = None


def build_nc() -> bass.Bass:
    nc = bass.Bass()
    x = nc.declare_dram_parameter("x", [BS, H, W, C], mybir.dt.float32, isOutput=False)
    y = nc.declare_dram_parameter(
        "y", [BS, H * KS, W * KS, OC], mybir.dt.float32, isOutput=True
    )

    # src[:, i, :]: [[256, BS*H*W], [1, 128]] starting at element offset i*128
    src = x.rearrange("b h w (i jc) -> (b h w) i jc", i=KS)
    # dst[:, i, :]: [[16384, BS*H], [1, 8192]] starting at element offset i*8192
    dst = y.rearrange("b (h i) w c -> (b h) i (w c)", i=KS)
    n_rows = BS * H  # 256
    n_src = BS * H * W  # 16384

    K, D = K_CHUNKS, LOOKAHEAD
    cs, cr = n_src // K, n_rows // K

    with nc.Block() as block:
        sp_sems = [nc.alloc_semaphore(f"sp_c{c}") for c in range(K)]
        act_sems = [nc.alloc_semaphore(f"act_c{c}") for c in range(K)]

        def stream(eng, i, own_sems, other_sems):
            for s in own_sems:
                eng.sem_clear(s)
            for c in range(K):
                if c - D >= 0:
                    eng.wait_ge(other_sems[c - D], 16)
                eng.dma_start(
                    out=dst[c * cr : (c + 1) * cr, i, :],
                    in_=src[c * cs : (c + 1) * cs, i, :],
                ).then_inc(own_sems[c], 16)
            for s in own_sems + other_sems:
                eng.wait_ge(s, 16)

        @block.sync
        def _(sync):
            stream(sync, 0, sp_sems, act_sems)

        @block.scalar
        def _(act):
            stream(act, 1, act_sems, sp_sems)

    return nc


def kernel(batch: np.ndarray) -> np.ndarray:
    global _nc_cache
    if _nc_cache is None:
        _nc_cache = build_nc()
    nc = _nc_cache

    batch = np.ascontiguousarray(np.asarray(batch), dtype=np.float32)
    assert batch.shape == (B, H, W, C), batch.shape

    in_maps = [{"x": batch[k * BS : (k + 1) * BS]} for k in range(N_CORES)]
    res = run_bass_kernel_spmd(nc, in_maps, list(range(N_CORES)))
    return np.concatenate([res.results[k]["y"] for k in range(N_CORES)], axis=0)
